# revision 10
# baseline (speedup 1.0000x reference)
"""Multi-head self-attention (pre-LN, residual) Trainium2 Bass kernel.

Problem: B=4, S=2048, D=128, H=4, Dh=32, fp32.
Sharding: 8 cores = 4 batches x 2 query-halves (1024 queries/core).

Host precomputes LayerNorm in fp64 and ships xn^T [d, s] bf16 directly
(plus raw x^T bf16 for the residual), so the device does no stats/
normalize/transpose work.  gamma/ISQ are folded into the projection
weights, beta/biases into bq_eff/bk_eff/rbias (rbias absorbs
Wo.T@bv_eff); vecs/woAB ship pre-transposed so no DMA rearranges.

Device dataflow per core ([feature, seq] layouts):
  kT/qT/v: QKV projections from xnT (PE matmuls via a 2-bank util psum
  ring; K/Q unloads on ACT with bias, V unload on DVE).  V per head is
  a 64-col block [ones(den), v(32), zeros(31)].

  Attention loop (2 q-chunks x 16 k-tiles), per iteration:
    scores: 4 row-tiled MMs (tile_position=(32h,0)), one single-bank
            psum tile PER HEAD so the ACT and DVE exp pipelines stay
            decoupled and each engine's next-iteration score MM hides
            inside its other exp sub-op's window
    exp:    heads 0-1 on ACT ([128,512] Exp each, bias=-8), heads 2-3
            on DVE (Schraudolph tensor_scalar: int16(x*SA+SB) bits ==
            bf16 exp) -> pa/pb bf16 tiles
    ctx:    4 accumulating MMs, bank A = heads {0,1}, bank B = {2,3},
            M=64 col-tiles; row 64i = den (ones col of V)
  Projections for later chunks drip into the loop's engine slack.

  Tail per chunk: ctx unload (ACT/DVE bf16), den broadcast via masked
  bf16 matmul, DVE fast reciprocal, ctxn multiply (GPSIMD for the
  dripped chunk-0 tail), Wo as K=128 f32r matmuls, residual add (DVE),
  DMA out.  Chunk-0 tail drips into chunk-1's loop; the final tail is
  half-width pipelined with output DMAs spread across queues.

PSUM (8 banks): sH0-3 4 + ctxA/ctxB 2 + util ring 2 (projections, den
broadcast, output projection, HAM warm-up).
"""

import sys

if "/opt/trn_rl_repo" not in sys.path:
    sys.path.insert(0, "/opt/trn_rl_repo")

import numpy as np

import concourse.bacc as bacc
import concourse.tile as tile
import concourse.mybir as mybir
from concourse.bass_utils import run_bass_kernel_spmd
from concourse.masks import make_identity

F32 = mybir.dt.float32
F32R = mybir.dt.float32r
BF16 = mybir.dt.bfloat16
I16 = mybir.dt.int16
AF = mybir.ActivationFunctionType
OP = mybir.AluOpType

B, S, D = 4, 2048, 128
H, DH = 4, 32
N_CORES = 8
QH = S // 2  # queries per core
CHUNK = 512
NKT = S // 128  # 16 k-tiles
EPS = 1e-6
SHIFT = 8.0
ISQ = 1.0 / np.sqrt(np.float32(DH))
# Schraudolph bf16 exp: int16(x*SA + SB).bits == bf16(exp(x - SHIFT))
SA = float(128.0 / np.log(2.0))
SB = float(127.0 * 128.0 - 0.0579 * 128.0 - SHIFT * 128.0 / np.log(2.0))

_compiled = None


def _build():
    nc = bacc.Bacc(
        "TRN2",
        target_bir_lowering=False,
        debug=False,
        enable_asserts=False,
        num_devices=N_CORES,
    )

    xnT_d = nc.dram_tensor("xnT", [D, S], BF16, kind="ExternalInput").ap()
    xt_d = nc.dram_tensor("xt", [D, QH], BF16, kind="ExternalInput").ap()
    wq_d = nc.dram_tensor("wq", [D, D], BF16, kind="ExternalInput").ap()
    wk_d = nc.dram_tensor("wk", [D, D], BF16, kind="ExternalInput").ap()
    wv_d = nc.dram_tensor("wv", [D, D], BF16, kind="ExternalInput").ap()
    # pre-transposed on host: [d, group, e]; group 0 rows {1-33: h0,
    # 65-97: h1}, group 1 {1-33: h2, 65-97: h3}
    woAB_d = nc.dram_tensor("woAB", [D, 2, D], F32R, kind="ExternalInput").ap()
    # pre-transposed on host: cols bq_eff, bk_eff, rbias
    vecs_d = nc.dram_tensor("vecs", [D, 3], F32, kind="ExternalInput").ap()
    outT_d = nc.dram_tensor("outT", [D, QH], F32, kind="ExternalOutput").ap()

    with tile.TileContext(nc) as tc:
        consts = tc.alloc_tile_pool(name="consts", bufs=1)
        sbW = tc.alloc_tile_pool(name="sbW", bufs=1)
        sbBig = tc.alloc_tile_pool(name="sbBig", bufs=1)
        sbTmp = tc.alloc_tile_pool(name="sbTmp", bufs=3)
        pPool = tc.alloc_tile_pool(name="pPool", bufs=3)

        # tiny consts needed by the exp warm-up
        nshift = consts.tile([128, 1], F32)
        nc.vector.memset(nshift, -SHIFT)
        dummy = consts.tile([128, 1], F32)
        nc.vector.memset(dummy, 0.0)

        # ---- input DMAs first ----
        xnT_sb = sbBig.tile([128, S], BF16)
        xt_sb = sbBig.tile([128, QH], BF16)
        wq_f = sbW.tile([D, D], BF16)
        wk_f = sbW.tile([D, D], BF16)
        wv_f = sbW.tile([D, D], BF16)
        wo_sb = sbW.tile([D, 2, D], F32R)
        vecsT = sbW.tile([D, 3], F32)  # cols: bq_eff, bk_eff, rbias

        nc.sync.dma_start(out=xnT_sb[:, 0:1024], in_=xnT_d[:, 0:1024])
        nc.gpsimd.dma_start(out=xnT_sb[:, 1024:2048], in_=xnT_d[:, 1024:2048])
        nc.sync.dma_start(out=wk_f, in_=wk_d)
        nc.gpsimd.dma_start(out=wq_f, in_=wq_d)
        nc.sync.dma_start(out=vecsT, in_=vecs_d)
        nc.gpsimd.dma_start(out=wv_f, in_=wv_d)

        # force the exp table load early (hides ~2.7us in startup)
        warm_exp = sbTmp.tile([128, 1], F32, tag="we")
        nc.scalar.activation(warm_exp, dummy, AF.Exp, bias=nshift, scale=1.0)
        nc.scalar.dma_start(out=wo_sb, in_=woAB_d)
        nc.scalar.dma_start(out=xt_sb, in_=xt_d)

        # remaining consts
        wsrc = consts.tile([128, 512], BF16)
        nc.vector.memset(wsrc, 0.5)
        wones = consts.tile([128, DH], BF16)
        nc.vector.memset(wones, 1.0)
        msel = consts.tile([128, 128], BF16)
        nc.gpsimd.memset(msel, 0.0)
        nc.gpsimd.memset(msel[0:1, 0:64], 1.0)
        nc.gpsimd.memset(msel[64:65, 64:128], 1.0)
        identb = consts.tile([128, 128], BF16)
        make_identity(nc, identb)

        bqe = vecsT[:, 0:1]
        bke = vecsT[:, 1:2]
        rbias = vecsT[:, 2:3]

        # ---- PSUM: sH0-3 4 banks, ctxA/ctxB 1 each, util ring 2 ----
        ps = tc.alloc_tile_pool(name="ps", bufs=1, space="PSUM")

        def util_tile(name):
            return ps.tile([128, CHUNK], F32, name=name, tag="util", bufs=2)

        # ---- projections ----
        kT = sbBig.tile([128, S], BF16)
        qT = sbBig.tile([128, QH], BF16)
        # V per head: 64 cols = [ones (den), 32 v-dims, 31 zeros]
        v_sb = sbBig.tile([128, NKT, H, 64], BF16)
        nc.gpsimd.memset(v_sb[:, :, :, 33:64], 0.0)
        nc.gpsimd.memset(v_sb[:, :, :, 0:1], 1.0)
        residT = sbBig.tile([128, QH], BF16)  # x^T + rbias (query half)

        def kproj(c):
            u = util_tile("kp")
            nc.tensor.matmul(
                u, wk_f, xnT_sb[:, c * CHUNK : (c + 1) * CHUNK],
                start=True, stop=True,
            )
            nc.scalar.add(kT[:, c * CHUNK : (c + 1) * CHUNK], u, bke)

        def qproj(c):
            u = util_tile("qp")
            nc.tensor.matmul(
                u, wq_f, xnT_sb[:, c * CHUNK : (c + 1) * CHUNK],
                start=True, stop=True,
            )
            nc.scalar.add(qT[:, c * CHUNK : (c + 1) * CHUNK], u, bqe)

        def vproj(b4):
            u = util_tile("vp")
            for i, t in enumerate(range(b4 * 4, b4 * 4 + 4)):
                nc.tensor.matmul(
                    u[:, i * 128 : (i + 1) * 128],
                    xnT_sb[:, t * 128 : (t + 1) * 128],
                    wv_f,
                    start=True,
                    stop=True,
                )
            sl4 = slice(b4 * 4, b4 * 4 + 4)
            uv = u.rearrange("p (t h d) -> p t h d", t=4, h=4, d=32)
            nc.vector.tensor_copy(v_sb[:, sl4, :, 1:33], uv)

        def resid_add(half):
            sl = slice(half * CHUNK, (half + 1) * CHUNK)
            nc.vector.tensor_scalar_add(residT[:, sl], xt_sb[:, sl], rbias)

        # all projections upfront (startup DMA latency hides them); the
        # HAM warm-up chain rides the util ring BEHIND them so the
        # projections never wait on warm-up matmuls
        kproj(0)
        qproj(0)
        vproj(0)
        kproj(1)
        vproj(1)
        qproj(1)
        kproj(2)
        vproj(2)
        kproj(3)
        vproj(3)
        for _ in range(2):
            wps = util_tile("wps")
            nc.tensor.matmul(wps[0:DH, :], wones, wsrc, start=True, stop=True)

        # ---- attention ----
        ctx_ps = {}

        def attn_scores(qc, kt):
            q0 = qc * CHUNK
            k0 = kt * 128
            ss = []
            for h in range(H):
                s = ps.tile([128, CHUNK], F32, name=f"sH{h}", tag=f"sH{h}", bufs=1)
                nc.tensor.matmul(
                    s,
                    kT[h * DH : (h + 1) * DH, k0 : k0 + 128],
                    qT[h * DH : (h + 1) * DH, q0 : q0 + CHUNK],
                    start=True,
                    stop=True,
                    tile_position=(h * DH, 0),
                )
                ss.append(s)
            return ss

        def attn_exp(ss):
            pa = pPool.tile([128, 2 * CHUNK], BF16, tag="pa")
            for i in range(2):
                sl = slice(i * CHUNK, (i + 1) * CHUNK)
                nc.scalar.activation(
                    pa[:, sl], ss[i], AF.Exp, bias=nshift, scale=1.0
                )
            pb = pPool.tile([128, 2 * CHUNK], I16, tag="pb")
            for i in range(2):
                sl = slice(i * CHUNK, (i + 1) * CHUNK)
                nc.vector.tensor_scalar(
                    pb[:, sl], ss[2 + i], SA, SB, op0=OP.mult, op1=OP.add
                )
            return pa, pb.bitcast(BF16)

        def attn_ctx(kt, p):
            pa, pb = p
            first, last = kt == 0, kt == NKT - 1
            for g, (bank, psrc) in enumerate((("A", pa), ("B", pb))):
                for i in range(2):
                    nc.tensor.matmul(
                        ctx_ps[bank][64 * i : 64 * i + 64, :],
                        v_sb[:, kt, 2 * g + i, :],
                        psrc[:, i * CHUNK : (i + 1) * CHUNK],
                        start=first,
                        stop=last,
                        tile_position=(0, 64 * i),
                        skip_group_check=True,
                    )

        # ---- tails ----
        tail_state = {}

        def tail_copy(qc):
            st = {}
            csA = sbTmp.tile([128, CHUNK], BF16, tag="csA")
            nc.scalar.copy(csA, ctx_ps["A"])
            st["A"] = csA
            csB = sbTmp.tile([128, CHUNK], BF16, tag="csB")
            nc.vector.tensor_copy(csB, ctx_ps["B"])
            st["B"] = csB
            tail_state[qc] = st

        def tail_dps(qc, g):
            # den broadcast via masked bf16 matmul (rows 0/64 -> all rows)
            dps = util_tile(f"dps{g}")
            nc.tensor.matmul(dps, msel, tail_state[qc][g], start=True, stop=True)
            tail_state[qc][g + "d"] = dps

        def tail_recip(qc, g):
            dinv = sbTmp.tile([128, CHUNK], F32, tag=f"di{g}")
            nc.vector.reciprocal_approx_fast(dinv, tail_state[qc][g + "d"])
            tail_state[qc][g + "i"] = dinv

        def tail_mult(qc, g):
            ctxn = sbTmp.tile([128, CHUNK], F32R, tag=f"cn{g}")
            nc.gpsimd.tensor_tensor(
                ctxn, tail_state[qc][g], tail_state[qc][g + "i"], op=OP.mult
            )
            tail_state[qc][g + "n"] = ctxn

        def tail_out(qc):
            q0 = qc * CHUNK
            outp = util_tile("outp")
            for gi, g in enumerate(("A", "B")):
                nc.tensor.matmul(
                    outp,
                    wo_sb[:, gi, :],
                    tail_state[qc][g + "n"],
                    start=(gi == 0),
                    stop=False,
                )
            # residual added in psum via identity matmul (frees DVE)
            nc.tensor.matmul(
                outp, identb, residT[:, q0 : q0 + CHUNK],
                start=False, stop=True,
            )
            fin = sbTmp.tile([128, CHUNK], F32, tag="fin")
            nc.scalar.copy(fin, outp)
            nc.gpsimd.dma_start(out=outT_d[:, q0 : q0 + CHUNK], in_=fin)

        # ---- schedule ----
        ctx_ps = {
            "A": ps.tile([128, CHUNK], F32, name="ctxA0", tag="ctxA", bufs=1),
            "B": ps.tile([128, CHUNK], F32, name="ctxB0", tag="ctxB", bufs=1),
        }
        drip0 = {
            4: lambda: resid_add(0),
            6: lambda: resid_add(1),
        }
        ss = attn_scores(0, 0)
        pending = attn_exp(ss)
        for kt in range(NKT):
            if kt in drip0:
                drip0[kt]()
            if kt + 1 < NKT:
                ss = attn_scores(0, kt + 1)
                nxt = attn_exp(ss)
            else:
                nxt = None
            attn_ctx(kt, pending)
            pending = nxt

        tail_copy(0)

        # chunk 1: chunk-0 tail pieces interleaved
        ctx_ps = {
            "A": ps.tile([128, CHUNK], F32, name="ctxA1", tag="ctxA", bufs=1),
            "B": ps.tile([128, CHUNK], F32, name="ctxB1", tag="ctxB", bufs=1),
        }
        drip1 = {
            1: lambda: tail_dps(0, "A"),
            2: lambda: tail_recip(0, "A"),
            3: lambda: tail_mult(0, "A"),
            5: lambda: tail_dps(0, "B"),
            6: lambda: tail_recip(0, "B"),
            7: lambda: tail_mult(0, "B"),
            9: lambda: tail_out(0),
        }
        ss = attn_scores(1, 0)
        pending = attn_exp(ss)
        for kt in range(NKT):
            if kt in drip1:
                drip1[kt]()
            if kt + 1 < NKT:
                ss = attn_scores(1, kt + 1)
                nxt = attn_exp(ss)
            else:
                nxt = None
            attn_ctx(kt, pending)
            pending = nxt

        # chunk-1 endgame: half-width pipeline so the first out-DMA starts
        # early; output DMAs spread across queues
        q0 = CHUNK
        csA = sbTmp.tile([128, CHUNK], BF16, tag="csA")
        csB = sbTmp.tile([128, CHUNK], BF16, tag="csB")
        diA = sbTmp.tile([128, CHUNK], F32, tag="diA")
        diB = sbTmp.tile([128, CHUNK], F32, tag="diB")
        cnA = sbTmp.tile([128, CHUNK], F32R, tag="cnA")
        cnB = sbTmp.tile([128, CHUNK], F32R, tag="cnB")
        fin = sbTmp.tile([128, CHUNK], F32, tag="fin")
        dps = util_tile("dpsf")
        outp = util_tile("outpf")
        for hf in range(2):
            sl = slice(hf * 256, (hf + 1) * 256)
            nc.scalar.copy(csA[:, sl], ctx_ps["A"][:, sl])
            nc.scalar.copy(csB[:, sl], ctx_ps["B"][:, sl])
            d0 = hf * 256
            nc.tensor.matmul(
                dps[:, d0 : d0 + 256], msel, csA[:, sl], start=True, stop=True
            )
            nc.vector.reciprocal_approx_fast(diA[:, sl], dps[:, d0 : d0 + 256])
            nc.tensor.matmul(
                dps[:, d0 : d0 + 256], msel, csB[:, sl], start=True, stop=True
            )
            nc.vector.reciprocal_approx_fast(diB[:, sl], dps[:, d0 : d0 + 256])
            nc.vector.tensor_mul(cnA[:, sl], csA[:, sl], diA[:, sl])
            nc.vector.tensor_mul(cnB[:, sl], csB[:, sl], diB[:, sl])
            o0 = hf * 256
            nc.tensor.matmul(
                outp[:, o0 : o0 + 256], wo_sb[:, 0, :], cnA[:, sl],
                start=True, stop=False,
            )
            nc.tensor.matmul(
                outp[:, o0 : o0 + 256], wo_sb[:, 1, :], cnB[:, sl],
                start=False, stop=False,
            )
            nc.tensor.matmul(
                outp[:, o0 : o0 + 256],
                identb,
                residT[:, q0 + hf * 256 : q0 + (hf + 1) * 256],
                start=False,
                stop=True,
            )
            nc.scalar.copy(fin[:, sl], outp[:, o0 : o0 + 256])
            eng = nc.sync if hf == 0 else nc.gpsimd
            eng.dma_start(
                out=outT_d[:, q0 + hf * 256 : q0 + (hf + 1) * 256], in_=fin[:, sl]
            )

        pPool.release()
        ps.release()
        sbTmp.release()
        sbBig.release()
        sbW.release()
        consts.release()

    nc.compile()
    return nc


def _get_compiled():
    global _compiled
    if _compiled is None:
        _compiled = _build()
    return _compiled


def kernel(x, Wq, bq, Wk, bk, Wv, bv, gamma, beta, Wo, bo):
    bf16 = mybir.dt.np(BF16)
    x = np.asarray(x, dtype=np.float64)
    Wq = np.asarray(Wq, dtype=np.float64)
    Wk = np.asarray(Wk, dtype=np.float64)
    Wv = np.asarray(Wv, dtype=np.float64)
    Wo = np.asarray(Wo, dtype=np.float64)
    gamma = np.asarray(gamma, dtype=np.float64)
    beta = np.asarray(beta, dtype=np.float64)
    bq = np.asarray(bq, dtype=np.float64)
    bk = np.asarray(bk, dtype=np.float64)
    bv = np.asarray(bv, dtype=np.float64)
    bo = np.asarray(bo, dtype=np.float64)

    # fold gamma (and ISQ into q) into the projections; beta into biases
    wq_f = np.ascontiguousarray((Wq * gamma[:, None] * ISQ).astype(bf16))
    wk_f = np.ascontiguousarray((Wk * gamma[:, None]).astype(bf16))
    wv_f = np.ascontiguousarray((Wv * gamma[:, None]).astype(bf16))
    bq_eff = (Wq.T @ beta + bq) * ISQ
    bk_eff = Wk.T @ beta + bk
    bv_eff = Wv.T @ beta + bv
    rbias = Wo.T @ bv_eff + bo

    # Wo rows permuted to the 2-bank ctx layout, shipped pre-transposed
    # as [d, group, e]: group 0 = {h0 at rows 1-33, h1 at 65-97},
    # group 1 = {h2, h3}
    woAB = np.zeros((2, D, D), dtype=np.float64)
    woAB[0, 1:33] = Wo[0 * DH : 1 * DH]
    woAB[0, 65:97] = Wo[1 * DH : 2 * DH]
    woAB[1, 1:33] = Wo[2 * DH : 3 * DH]
    woAB[1, 65:97] = Wo[3 * DH : 4 * DH]
    woAB_t = np.ascontiguousarray(woAB.transpose(1, 0, 2).astype(np.float32))

    vecs_t = np.ascontiguousarray(
        np.stack([bq_eff, bk_eff, rbias], axis=1).astype(np.float32)
    )

    # host LayerNorm in fp64 (gamma/beta folded into weights/biases above)
    mu = x.mean(axis=-1, keepdims=True)
    var = np.square(x - mu).mean(axis=-1, keepdims=True)
    xn = (x - mu) / np.sqrt(var + EPS)  # [B, S, D]

    nc = _get_compiled()

    in_maps = []
    for c in range(N_CORES):
        b, half = c // 2, c % 2
        off = half * QH
        xn_roll = np.roll(xn[b], -off, axis=0)
        x_roll = np.roll(x[b], -off, axis=0)
        in_maps.append(
            {
                "xnT": np.ascontiguousarray(xn_roll.T.astype(bf16)),
                "xt": np.ascontiguousarray(x_roll[0:QH].T.astype(bf16)),
                "wq": wq_f,
                "wk": wk_f,
                "wv": wv_f,
                "woAB": woAB_t,
                "vecs": vecs_t,
            }
        )

    res = run_bass_kernel_spmd(nc, in_maps, core_ids=list(range(N_CORES)), trace=False)

    out = np.empty((B, S, D), dtype=np.float32)
    for c in range(N_CORES):
        b, half = c // 2, c % 2
        off = half * QH
        out[b, off : off + QH, :] = res.results[c]["outT"].T
    return out


# revision 11
# speedup vs baseline: 1.0850x; 1.0850x over previous
"""Multi-head self-attention (pre-LN, residual) Trainium2 Bass kernel.

Problem: B=4, S=2048, D=128, H=4, Dh=32, fp32.
Sharding: 8 cores = 4 batches x 2 query-halves (1024 queries/core).

Host precomputes LayerNorm in fp64 and ships xn^T [d, s] bf16 directly
(plus raw x^T bf16 for the residual), so the device does no stats/
normalize/transpose work.  gamma/ISQ are folded into the projection
weights, beta/biases into bq_eff/bk_eff/rbias (rbias absorbs
Wo.T@bv_eff); vecs/woAB ship pre-transposed so no DMA rearranges.

Device dataflow per core ([feature, seq] layouts):
  kT/qT/v: QKV projections from xnT (PE matmuls via a 2-bank util psum
  ring; K/Q unloads on ACT with bias, V unload on DVE).  V per head is
  a 64-col block [ones(den), v(32), zeros(31)].

  Attention loop (2 q-chunks x 16 k-tiles), per iteration:
    scores: 4 row-tiled MMs (tile_position=(32h,0)), one single-bank
            psum tile PER HEAD so the ACT and DVE exp pipelines stay
            decoupled and each engine's next-iteration score MM hides
            inside its other exp sub-op's window
    exp:    heads 0-1 on ACT ([128,512] Exp each, bias=-8), heads 2-3
            on DVE (Schraudolph tensor_scalar: int16(x*SA+SB) bits ==
            bf16 exp) -> pa/pb bf16 tiles
    ctx:    4 accumulating MMs, bank A = heads {0,1}, bank B = {2,3},
            M=64 col-tiles; row 64i = den (ones col of V)
  Projections for later chunks drip into the loop's engine slack.

  Tail per chunk: ctx unload (ACT/DVE bf16), den broadcast via masked
  bf16 matmul, DVE fast reciprocal, ctxn multiply (GPSIMD for the
  dripped chunk-0 tail), Wo as K=128 f32r matmuls, residual add (DVE),
  DMA out.  Chunk-0 tail drips into chunk-1's loop; the final tail is
  half-width pipelined with output DMAs spread across queues.

PSUM (8 banks): sH0-3 4 + ctxA/ctxB 2 + util ring 2 (projections, den
broadcast, output projection, HAM warm-up).
"""

import sys

if "/opt/trn_rl_repo" not in sys.path:
    sys.path.insert(0, "/opt/trn_rl_repo")

import numpy as np

import concourse.bacc as bacc
import concourse.tile as tile
import concourse.mybir as mybir
from concourse.bass_utils import run_bass_kernel_spmd
from concourse.masks import make_identity

F32 = mybir.dt.float32
F32R = mybir.dt.float32r
BF16 = mybir.dt.bfloat16
I16 = mybir.dt.int16
AF = mybir.ActivationFunctionType
OP = mybir.AluOpType

B, S, D = 4, 2048, 128
H, DH = 4, 32
N_CORES = 8
QH = S // 2  # queries per core
CHUNK = 512
NKT = S // 128  # 16 k-tiles
EPS = 1e-6
SHIFT = 8.0
ISQ = 1.0 / np.sqrt(np.float32(DH))
# Schraudolph bf16 exp: int16(x*SA + SB).bits == bf16(exp(x - SHIFT))
SA = float(128.0 / np.log(2.0))
SB = float(127.0 * 128.0 - 0.0579 * 128.0 - SHIFT * 128.0 / np.log(2.0))

_compiled = None


def _build():
    nc = bacc.Bacc(
        "TRN2",
        target_bir_lowering=False,
        debug=False,
        enable_asserts=False,
        num_devices=N_CORES,
    )

    xnT_d = nc.dram_tensor("xnT", [D, S], BF16, kind="ExternalInput").ap()
    xt_d = nc.dram_tensor("xt", [D, QH], BF16, kind="ExternalInput").ap()
    wq_d = nc.dram_tensor("wq", [D, D], BF16, kind="ExternalInput").ap()
    wk_d = nc.dram_tensor("wk", [D, D], BF16, kind="ExternalInput").ap()
    wv_d = nc.dram_tensor("wv", [D, D], BF16, kind="ExternalInput").ap()
    # pre-transposed on host: [d, group, e]; group 0 rows {1-33: h0,
    # 65-97: h1}, group 1 {1-33: h2, 65-97: h3}
    woAB_d = nc.dram_tensor("woAB", [D, 2, D], F32R, kind="ExternalInput").ap()
    # pre-transposed on host: cols bq_eff, bk_eff, rbias
    vecs_d = nc.dram_tensor("vecs", [D, 3], F32, kind="ExternalInput").ap()
    outT_d = nc.dram_tensor("outT", [D, QH], F32, kind="ExternalOutput").ap()

    with tile.TileContext(nc) as tc:
        consts = tc.alloc_tile_pool(name="consts", bufs=1)
        sbW = tc.alloc_tile_pool(name="sbW", bufs=1)
        sbBig = tc.alloc_tile_pool(name="sbBig", bufs=1)
        sbTmp = tc.alloc_tile_pool(name="sbTmp", bufs=3)
        pPool = tc.alloc_tile_pool(name="pPool", bufs=3)

        # tiny consts needed by the exp warm-up
        nshift = consts.tile([128, 1], F32)
        nc.vector.memset(nshift, -SHIFT)
        dummy = consts.tile([128, 1], F32)
        nc.vector.memset(dummy, 0.0)

        # ---- input DMAs first ----
        xnT_sb = sbBig.tile([128, S], BF16)
        xt_sb = sbBig.tile([128, QH], BF16)
        wq_f = sbW.tile([D, D], BF16)
        wk_f = sbW.tile([D, D], BF16)
        wv_f = sbW.tile([D, D], BF16)
        wo_sb = sbW.tile([D, 2, D], F32R)
        vecsT = sbW.tile([D, 3], F32)  # cols: bq_eff, bk_eff, rbias

        nc.sync.dma_start(out=xnT_sb[:, 0:1024], in_=xnT_d[:, 0:1024])
        nc.gpsimd.dma_start(out=xnT_sb[:, 1024:2048], in_=xnT_d[:, 1024:2048])
        nc.sync.dma_start(out=wk_f, in_=wk_d)
        nc.gpsimd.dma_start(out=wq_f, in_=wq_d)
        nc.sync.dma_start(out=vecsT, in_=vecs_d)
        nc.gpsimd.dma_start(out=wv_f, in_=wv_d)

        # force the exp table load early (hides ~2.7us in startup)
        warm_exp = sbTmp.tile([128, 1], F32, tag="we")
        nc.scalar.activation(warm_exp, dummy, AF.Exp, bias=nshift, scale=1.0)
        nc.scalar.dma_start(out=wo_sb, in_=woAB_d)
        nc.scalar.dma_start(out=xt_sb, in_=xt_d)

        # remaining consts
        wsrc = consts.tile([128, 512], BF16)
        nc.vector.memset(wsrc, 0.5)
        wones = consts.tile([128, DH], BF16)
        nc.vector.memset(wones, 1.0)
        msel = consts.tile([128, 128], BF16)
        nc.gpsimd.memset(msel, 0.0)
        nc.gpsimd.memset(msel[0:1, 0:64], 1.0)
        nc.gpsimd.memset(msel[64:65, 64:128], 1.0)
        identb = consts.tile([128, 128], BF16)
        make_identity(nc, identb)

        bqe = vecsT[:, 0:1]
        bke = vecsT[:, 1:2]
        rbias = vecsT[:, 2:3]

        # ---- PSUM: sH0-3 4 banks, ctxA/ctxB 1 each, util ring 2 ----
        ps = tc.alloc_tile_pool(name="ps", bufs=1, space="PSUM")

        def util_tile(name):
            return ps.tile([128, CHUNK], F32, name=name, tag="util", bufs=2)

        # ---- projections ----
        kT = sbBig.tile([128, S], BF16)
        qT = sbBig.tile([128, QH], BF16)
        # V per head: 64 cols = [ones (den), 32 v-dims, 31 zeros]
        v_sb = sbBig.tile([128, NKT, H, 64], BF16)
        nc.gpsimd.memset(v_sb[:, :, :, 33:64], 0.0)
        nc.gpsimd.memset(v_sb[:, :, :, 0:1], 1.0)
        residT = sbBig.tile([128, QH], BF16)  # x^T + rbias (query half)

        def kproj(c):
            u = util_tile("kp")
            nc.tensor.matmul(
                u, wk_f, xnT_sb[:, c * CHUNK : (c + 1) * CHUNK],
                start=True, stop=True,
            )
            nc.scalar.add(kT[:, c * CHUNK : (c + 1) * CHUNK], u, bke)

        def qproj(c):
            u = util_tile("qp")
            nc.tensor.matmul(
                u, wq_f, xnT_sb[:, c * CHUNK : (c + 1) * CHUNK],
                start=True, stop=True,
            )
            nc.scalar.add(qT[:, c * CHUNK : (c + 1) * CHUNK], u, bqe)

        def vproj(b4):
            u = util_tile("vp")
            for i, t in enumerate(range(b4 * 4, b4 * 4 + 4)):
                nc.tensor.matmul(
                    u[:, i * 128 : (i + 1) * 128],
                    xnT_sb[:, t * 128 : (t + 1) * 128],
                    wv_f,
                    start=True,
                    stop=True,
                )
            sl4 = slice(b4 * 4, b4 * 4 + 4)
            uv = u.rearrange("p (t h d) -> p t h d", t=4, h=4, d=32)
            nc.vector.tensor_copy(v_sb[:, sl4, :, 1:33], uv)

        def resid_add(half):
            sl = slice(half * CHUNK, (half + 1) * CHUNK)
            nc.vector.tensor_scalar_add(residT[:, sl], xt_sb[:, sl], rbias)

        # chunk-0 projections upfront; the HAM warm-up chain rides the
        # util ring BEHIND them so projections never wait on warm-ups
        kproj(0)
        qproj(0)
        vproj(0)
        for _ in range(2):
            wps = util_tile("wps")
            nc.tensor.matmul(wps[0:DH, :], wones, wsrc, start=True, stop=True)

        # ---- attention ----
        ctx_ps = {}

        def attn_scores(qc, kt):
            q0 = qc * CHUNK
            k0 = kt * 128
            ss = []
            for h in range(H):
                s = ps.tile([128, CHUNK], F32, name=f"sH{h}", tag=f"sH{h}", bufs=1)
                nc.tensor.matmul(
                    s,
                    kT[h * DH : (h + 1) * DH, k0 : k0 + 128],
                    qT[h * DH : (h + 1) * DH, q0 : q0 + CHUNK],
                    start=True,
                    stop=True,
                    tile_position=(h * DH, 0),
                )
                ss.append(s)
            return ss

        def attn_exp(ss):
            pa = pPool.tile([128, 2 * CHUNK], BF16, tag="pa")
            for i in range(2):
                sl = slice(i * CHUNK, (i + 1) * CHUNK)
                nc.scalar.activation(
                    pa[:, sl], ss[i], AF.Exp, bias=nshift, scale=1.0
                )
            pb = pPool.tile([128, 2 * CHUNK], I16, tag="pb")
            for i in range(2):
                sl = slice(i * CHUNK, (i + 1) * CHUNK)
                nc.vector.tensor_scalar(
                    pb[:, sl], ss[2 + i], SA, SB, op0=OP.mult, op1=OP.add
                )
            return pa, pb.bitcast(BF16)

        def attn_ctx(kt, p):
            pa, pb = p
            first, last = kt == 0, kt == NKT - 1
            for g, (bank, psrc) in enumerate((("A", pa), ("B", pb))):
                for i in range(2):
                    nc.tensor.matmul(
                        ctx_ps[bank][64 * i : 64 * i + 64, :],
                        v_sb[:, kt, 2 * g + i, :],
                        psrc[:, i * CHUNK : (i + 1) * CHUNK],
                        start=first,
                        stop=last,
                        tile_position=(0, 64 * i),
                        skip_group_check=True,
                    )

        # ---- tails ----
        tail_state = {}

        def tail_copy(qc):
            st = {}
            csA = sbTmp.tile([128, CHUNK], BF16, tag="csA")
            nc.scalar.copy(csA, ctx_ps["A"])
            st["A"] = csA
            csB = sbTmp.tile([128, CHUNK], BF16, tag="csB")
            nc.vector.tensor_copy(csB, ctx_ps["B"])
            st["B"] = csB
            tail_state[qc] = st

        def tail_dps(qc, g):
            # den broadcast via masked bf16 matmul (rows 0/64 -> all rows)
            dps = util_tile(f"dps{g}")
            nc.tensor.matmul(dps, msel, tail_state[qc][g], start=True, stop=True)
            tail_state[qc][g + "d"] = dps

        def tail_recip(qc, g):
            dinv = sbTmp.tile([128, CHUNK], F32, tag=f"di{g}")
            nc.vector.reciprocal_approx_fast(dinv, tail_state[qc][g + "d"])
            tail_state[qc][g + "i"] = dinv

        def tail_mult(qc, g):
            ctxn = sbTmp.tile([128, CHUNK], F32R, tag=f"cn{g}")
            nc.gpsimd.tensor_tensor(
                ctxn, tail_state[qc][g], tail_state[qc][g + "i"], op=OP.mult
            )
            tail_state[qc][g + "n"] = ctxn

        def tail_out(qc):
            q0 = qc * CHUNK
            outp = util_tile("outp")
            for gi, g in enumerate(("A", "B")):
                nc.tensor.matmul(
                    outp,
                    wo_sb[:, gi, :],
                    tail_state[qc][g + "n"],
                    start=(gi == 0),
                    stop=False,
                )
            # residual added in psum via identity matmul (frees DVE)
            nc.tensor.matmul(
                outp, identb, residT[:, q0 : q0 + CHUNK],
                start=False, stop=True,
            )
            fin = sbTmp.tile([128, CHUNK], F32, tag="fin")
            nc.scalar.copy(fin, outp)
            nc.gpsimd.dma_start(out=outT_d[:, q0 : q0 + CHUNK], in_=fin)

        # ---- schedule ----
        ctx_ps = {
            "A": ps.tile([128, CHUNK], F32, name="ctxA0", tag="ctxA", bufs=1),
            "B": ps.tile([128, CHUNK], F32, name="ctxB0", tag="ctxB", bufs=1),
        }
        drip0 = {
            1: lambda: kproj(1),
            2: lambda: vproj(1),
            4: lambda: kproj(2),
            5: lambda: vproj(2),
            7: lambda: kproj(3),
            8: lambda: vproj(3),
            10: lambda: qproj(1),
            12: lambda: resid_add(0),
            13: lambda: resid_add(1),
        }
        ss = attn_scores(0, 0)
        pending = attn_exp(ss)
        for kt in range(NKT):
            if kt in drip0:
                drip0[kt]()
            if kt + 1 < NKT:
                ss = attn_scores(0, kt + 1)
                nxt = attn_exp(ss)
            else:
                nxt = None
            attn_ctx(kt, pending)
            pending = nxt

        tail_copy(0)

        # chunk 1: chunk-0 tail pieces interleaved
        ctx_ps = {
            "A": ps.tile([128, CHUNK], F32, name="ctxA1", tag="ctxA", bufs=1),
            "B": ps.tile([128, CHUNK], F32, name="ctxB1", tag="ctxB", bufs=1),
        }
        drip1 = {
            1: lambda: tail_dps(0, "A"),
            2: lambda: tail_recip(0, "A"),
            3: lambda: tail_mult(0, "A"),
            5: lambda: tail_dps(0, "B"),
            6: lambda: tail_recip(0, "B"),
            7: lambda: tail_mult(0, "B"),
            9: lambda: tail_out(0),
        }
        ss = attn_scores(1, 0)
        pending = attn_exp(ss)
        for kt in range(NKT):
            if kt in drip1:
                drip1[kt]()
            if kt + 1 < NKT:
                ss = attn_scores(1, kt + 1)
                nxt = attn_exp(ss)
            else:
                nxt = None
            attn_ctx(kt, pending)
            pending = nxt

        # chunk-1 endgame: half-width pipeline so the first out-DMA starts
        # early; output DMAs spread across queues
        q0 = CHUNK
        csA = sbTmp.tile([128, CHUNK], BF16, tag="csA")
        csB = sbTmp.tile([128, CHUNK], BF16, tag="csB")
        diA = sbTmp.tile([128, CHUNK], F32, tag="diA")
        diB = sbTmp.tile([128, CHUNK], F32, tag="diB")
        cnA = sbTmp.tile([128, CHUNK], F32R, tag="cnA")
        cnB = sbTmp.tile([128, CHUNK], F32R, tag="cnB")
        fin = sbTmp.tile([128, CHUNK], F32, tag="fin")
        dps = util_tile("dpsf")
        outp = util_tile("outpf")
        for hf in range(2):
            sl = slice(hf * 256, (hf + 1) * 256)
            nc.scalar.copy(csA[:, sl], ctx_ps["A"][:, sl])
            nc.scalar.copy(csB[:, sl], ctx_ps["B"][:, sl])
            d0 = hf * 256
            nc.tensor.matmul(
                dps[:, d0 : d0 + 256], msel, csA[:, sl], start=True, stop=True
            )
            nc.vector.reciprocal_approx_fast(diA[:, sl], dps[:, d0 : d0 + 256])
            nc.tensor.matmul(
                dps[:, d0 : d0 + 256], msel, csB[:, sl], start=True, stop=True
            )
            nc.vector.reciprocal_approx_fast(diB[:, sl], dps[:, d0 : d0 + 256])
            nc.vector.tensor_mul(cnA[:, sl], csA[:, sl], diA[:, sl])
            nc.vector.tensor_mul(cnB[:, sl], csB[:, sl], diB[:, sl])
            o0 = hf * 256
            nc.tensor.matmul(
                outp[:, o0 : o0 + 256], wo_sb[:, 0, :], cnA[:, sl],
                start=True, stop=False,
            )
            nc.tensor.matmul(
                outp[:, o0 : o0 + 256], wo_sb[:, 1, :], cnB[:, sl],
                start=False, stop=False,
            )
            nc.tensor.matmul(
                outp[:, o0 : o0 + 256],
                identb,
                residT[:, q0 + hf * 256 : q0 + (hf + 1) * 256],
                start=False,
                stop=True,
            )
            nc.scalar.copy(fin[:, sl], outp[:, o0 : o0 + 256])
            eng = nc.sync if hf == 0 else nc.gpsimd
            eng.dma_start(
                out=outT_d[:, q0 + hf * 256 : q0 + (hf + 1) * 256], in_=fin[:, sl]
            )

        pPool.release()
        ps.release()
        sbTmp.release()
        sbBig.release()
        sbW.release()
        consts.release()

    nc.compile()
    return nc


def _get_compiled():
    global _compiled
    if _compiled is None:
        _compiled = _build()
    return _compiled


def kernel(x, Wq, bq, Wk, bk, Wv, bv, gamma, beta, Wo, bo):
    bf16 = mybir.dt.np(BF16)
    x = np.asarray(x, dtype=np.float64)
    Wq = np.asarray(Wq, dtype=np.float64)
    Wk = np.asarray(Wk, dtype=np.float64)
    Wv = np.asarray(Wv, dtype=np.float64)
    Wo = np.asarray(Wo, dtype=np.float64)
    gamma = np.asarray(gamma, dtype=np.float64)
    beta = np.asarray(beta, dtype=np.float64)
    bq = np.asarray(bq, dtype=np.float64)
    bk = np.asarray(bk, dtype=np.float64)
    bv = np.asarray(bv, dtype=np.float64)
    bo = np.asarray(bo, dtype=np.float64)

    # fold gamma (and ISQ into q) into the projections; beta into biases
    wq_f = np.ascontiguousarray((Wq * gamma[:, None] * ISQ).astype(bf16))
    wk_f = np.ascontiguousarray((Wk * gamma[:, None]).astype(bf16))
    wv_f = np.ascontiguousarray((Wv * gamma[:, None]).astype(bf16))
    bq_eff = (Wq.T @ beta + bq) * ISQ
    bk_eff = Wk.T @ beta + bk
    bv_eff = Wv.T @ beta + bv
    rbias = Wo.T @ bv_eff + bo

    # Wo rows permuted to the 2-bank ctx layout, shipped pre-transposed
    # as [d, group, e]: group 0 = {h0 at rows 1-33, h1 at 65-97},
    # group 1 = {h2, h3}
    woAB = np.zeros((2, D, D), dtype=np.float64)
    woAB[0, 1:33] = Wo[0 * DH : 1 * DH]
    woAB[0, 65:97] = Wo[1 * DH : 2 * DH]
    woAB[1, 1:33] = Wo[2 * DH : 3 * DH]
    woAB[1, 65:97] = Wo[3 * DH : 4 * DH]
    woAB_t = np.ascontiguousarray(woAB.transpose(1, 0, 2).astype(np.float32))

    vecs_t = np.ascontiguousarray(
        np.stack([bq_eff, bk_eff, rbias], axis=1).astype(np.float32)
    )

    # host LayerNorm in fp64 (gamma/beta folded into weights/biases above)
    mu = x.mean(axis=-1, keepdims=True)
    var = np.square(x - mu).mean(axis=-1, keepdims=True)
    xn = (x - mu) / np.sqrt(var + EPS)  # [B, S, D]

    nc = _get_compiled()

    in_maps = []
    for c in range(N_CORES):
        b, half = c // 2, c % 2
        off = half * QH
        xn_roll = np.roll(xn[b], -off, axis=0)
        x_roll = np.roll(x[b], -off, axis=0)
        in_maps.append(
            {
                "xnT": np.ascontiguousarray(xn_roll.T.astype(bf16)),
                "xt": np.ascontiguousarray(x_roll[0:QH].T.astype(bf16)),
                "wq": wq_f,
                "wk": wk_f,
                "wv": wv_f,
                "woAB": woAB_t,
                "vecs": vecs_t,
            }
        )

    res = run_bass_kernel_spmd(nc, in_maps, core_ids=list(range(N_CORES)), trace=False)

    out = np.empty((B, S, D), dtype=np.float32)
    for c in range(N_CORES):
        b, half = c // 2, c % 2
        off = half * QH
        out[b, off : off + QH, :] = res.results[c]["outT"].T
    return out


# revision 12
# speedup vs baseline: 1.1148x; 1.0275x over previous
"""Multi-head self-attention (pre-LN, residual) Trainium2 Bass kernel.

Problem: B=4, S=2048, D=128, H=4, Dh=32, fp32.
Sharding: 8 cores = 4 batches x 2 query-halves (1024 queries/core).

Host precomputes LayerNorm in fp64 and ships xn^T [d, s] bf16 directly
(plus raw x^T bf16 for the residual), so the device does no stats/
normalize/transpose work.  gamma/ISQ are folded into the projection
weights, beta/biases into bq_eff/bk_eff/rbias (rbias absorbs
Wo.T@bv_eff); vecs/woAB ship pre-transposed so no DMA rearranges.

Device dataflow per core ([feature, seq] layouts):
  kT/qT/v: QKV projections from xnT (PE matmuls via a 2-bank util psum
  ring; K/Q unloads on ACT with bias, V unload on DVE).  V per head is
  a 64-col block [ones(den), v(32), zeros(31)].

  Attention loop (2 q-chunks x 16 k-tiles), per iteration:
    scores: 4 row-tiled MMs (tile_position=(32h,0)), one single-bank
            psum tile PER HEAD so the ACT and DVE exp pipelines stay
            decoupled and each engine's next-iteration score MM hides
            inside its other exp sub-op's window
    exp:    heads 0-1 on ACT ([128,512] Exp each, bias=-8), heads 2-3
            on DVE (Schraudolph tensor_scalar: int16(x*SA+SB) bits ==
            bf16 exp) -> pa/pb bf16 tiles
    ctx:    4 accumulating MMs, bank A = heads {0,1}, bank B = {2,3},
            M=64 col-tiles; row 64i = den (ones col of V)
  Projections for later chunks drip into the loop's engine slack.

  Tail per chunk: ctx unload (ACT/DVE bf16), den broadcast via masked
  bf16 matmul, DVE fast reciprocal, ctxn multiply (GPSIMD for the
  dripped chunk-0 tail), Wo as K=128 f32r matmuls, residual add (DVE),
  DMA out.  Chunk-0 tail drips into chunk-1's loop; the final tail is
  half-width pipelined with output DMAs spread across queues.

PSUM (8 banks): sH0-3 4 + ctxA/ctxB 2 + util ring 2 (projections, den
broadcast, output projection, HAM warm-up).
"""

import sys

if "/opt/trn_rl_repo" not in sys.path:
    sys.path.insert(0, "/opt/trn_rl_repo")

import numpy as np

import concourse.bacc as bacc
import concourse.tile as tile
import concourse.mybir as mybir
from concourse.bass_utils import run_bass_kernel_spmd
from concourse.masks import make_identity

F32 = mybir.dt.float32
F32R = mybir.dt.float32r
BF16 = mybir.dt.bfloat16
I16 = mybir.dt.int16
AF = mybir.ActivationFunctionType
OP = mybir.AluOpType

B, S, D = 4, 2048, 128
H, DH = 4, 32
N_CORES = 8
QH = S // 2  # queries per core
CHUNK = 512
NKT = S // 128  # 16 k-tiles
EPS = 1e-6
SHIFT = 8.0
ISQ = 1.0 / np.sqrt(np.float32(DH))
# Schraudolph bf16 exp: int16(x*SA + SB).bits == bf16(exp(x - SHIFT))
SA = float(128.0 / np.log(2.0))
SB = float(127.0 * 128.0 - 0.0579 * 128.0 - SHIFT * 128.0 / np.log(2.0))

_compiled = None


def _build():
    nc = bacc.Bacc(
        "TRN2",
        target_bir_lowering=False,
        debug=False,
        enable_asserts=False,
        num_devices=N_CORES,
    )

    xnT_d = nc.dram_tensor("xnT", [D, S], BF16, kind="ExternalInput").ap()
    xt_d = nc.dram_tensor("xt", [D, QH], BF16, kind="ExternalInput").ap()
    wq_d = nc.dram_tensor("wq", [D, D], BF16, kind="ExternalInput").ap()
    wk_d = nc.dram_tensor("wk", [D, D], BF16, kind="ExternalInput").ap()
    wv_d = nc.dram_tensor("wv", [D, D], BF16, kind="ExternalInput").ap()
    # pre-transposed on host: [d, group, e]; group 0 rows {1-33: h0,
    # 65-97: h1}, group 1 {1-33: h2, 65-97: h3}
    woAB_d = nc.dram_tensor("woAB", [D, 2, D], F32R, kind="ExternalInput").ap()
    # pre-transposed on host: cols bq_eff, bk_eff, rbias
    vecs_d = nc.dram_tensor("vecs", [D, 3], F32, kind="ExternalInput").ap()
    outT_d = nc.dram_tensor("outT", [D, QH], F32, kind="ExternalOutput").ap()

    with tile.TileContext(nc) as tc:
        consts = tc.alloc_tile_pool(name="consts", bufs=1)
        sbW = tc.alloc_tile_pool(name="sbW", bufs=1)
        sbBig = tc.alloc_tile_pool(name="sbBig", bufs=1)
        sbTmp = tc.alloc_tile_pool(name="sbTmp", bufs=3)
        pPool = tc.alloc_tile_pool(name="pPool", bufs=3)

        # tiny consts needed by the exp warm-up
        nshift = consts.tile([128, 1], F32)
        nc.vector.memset(nshift, -SHIFT)
        dummy = consts.tile([128, 1], F32)
        nc.vector.memset(dummy, 0.0)

        # ---- input DMAs first ----
        xnT_sb = sbBig.tile([128, S], BF16)
        xt_sb = sbBig.tile([128, QH], BF16)
        wq_f = sbW.tile([D, D], BF16)
        wk_f = sbW.tile([D, D], BF16)
        wv_f = sbW.tile([D, D], BF16)
        wo_sb = sbW.tile([D, 2, D], F32R)
        vecsT = sbW.tile([D, 3], F32)  # cols: bq_eff, bk_eff, rbias

        nc.sync.dma_start(out=xnT_sb[:, 0:1024], in_=xnT_d[:, 0:1024])
        nc.gpsimd.dma_start(out=xnT_sb[:, 1024:2048], in_=xnT_d[:, 1024:2048])
        nc.sync.dma_start(out=wk_f, in_=wk_d)
        nc.gpsimd.dma_start(out=wq_f, in_=wq_d)
        nc.sync.dma_start(out=vecsT, in_=vecs_d)
        nc.gpsimd.dma_start(out=wv_f, in_=wv_d)

        # force the exp table load early (hides ~2.7us in startup)
        warm_exp = sbTmp.tile([128, 1], F32, tag="we")
        nc.scalar.activation(warm_exp, dummy, AF.Exp, bias=nshift, scale=1.0)
        nc.scalar.dma_start(out=wo_sb, in_=woAB_d)
        nc.scalar.dma_start(out=xt_sb, in_=xt_d)

        # remaining consts
        wsrc = consts.tile([128, 512], BF16)
        nc.vector.memset(wsrc, 0.5)
        wones = consts.tile([128, DH], BF16)
        nc.vector.memset(wones, 1.0)
        msel = consts.tile([128, 128], BF16)
        nc.gpsimd.memset(msel, 0.0)
        nc.gpsimd.memset(msel[0:1, 0:64], 1.0)
        nc.gpsimd.memset(msel[64:65, 64:128], 1.0)
        identb = consts.tile([128, 128], BF16)
        make_identity(nc, identb)

        bqe = vecsT[:, 0:1]
        bke = vecsT[:, 1:2]
        rbias = vecsT[:, 2:3]

        # ---- PSUM: sH0-3 4 banks, ctxA/ctxB 1 each, util ring 2 ----
        ps = tc.alloc_tile_pool(name="ps", bufs=1, space="PSUM")

        def util_tile(name):
            return ps.tile([128, CHUNK], F32, name=name, tag="util", bufs=2)

        # ---- projections ----
        kT = sbBig.tile([128, S], BF16)
        qT = sbBig.tile([128, QH], BF16)
        # V per head: 64 cols = [ones (den), 32 v-dims, 31 zeros]
        v_sb = sbBig.tile([128, NKT, H, 64], BF16)
        nc.gpsimd.memset(v_sb[:, :, :, 33:64], 0.0)
        nc.gpsimd.memset(v_sb[:, :, :, 0:1], 1.0)
        residT = sbBig.tile([128, QH], BF16)  # x^T + rbias (query half)

        def kproj(c):
            u = util_tile("kp")
            nc.tensor.matmul(
                u, wk_f, xnT_sb[:, c * CHUNK : (c + 1) * CHUNK],
                start=True, stop=True,
            )
            nc.scalar.add(kT[:, c * CHUNK : (c + 1) * CHUNK], u, bke)

        def qproj(c):
            u = util_tile("qp")
            nc.tensor.matmul(
                u, wq_f, xnT_sb[:, c * CHUNK : (c + 1) * CHUNK],
                start=True, stop=True,
            )
            nc.scalar.add(qT[:, c * CHUNK : (c + 1) * CHUNK], u, bqe)

        def vproj(b4):
            u = util_tile("vp")
            for i, t in enumerate(range(b4 * 4, b4 * 4 + 4)):
                nc.tensor.matmul(
                    u[:, i * 128 : (i + 1) * 128],
                    xnT_sb[:, t * 128 : (t + 1) * 128],
                    wv_f,
                    start=True,
                    stop=True,
                )
            sl4 = slice(b4 * 4, b4 * 4 + 4)
            uv = u.rearrange("p (t h d) -> p t h d", t=4, h=4, d=32)
            nc.vector.tensor_copy(v_sb[:, sl4, :, 1:33], uv)

        def resid_add(half):
            sl = slice(half * CHUNK, (half + 1) * CHUNK)
            nc.vector.tensor_scalar_add(residT[:, sl], xt_sb[:, sl], rbias)

        # chunk-0 projections upfront; the HAM warm-up chain rides the
        # util ring BEHIND them so projections never wait on warm-ups
        kproj(0)
        qproj(0)
        vproj(0)
        for _ in range(2):
            wps = util_tile("wps")
            nc.tensor.matmul(wps[0:DH, :], wones, wsrc, start=True, stop=True)

        # ---- attention ----
        ctx_ps = {}

        def attn_scores(qc, kt):
            q0 = qc * CHUNK
            k0 = kt * 128
            ss = []
            for h in range(H):
                s = ps.tile([128, CHUNK], F32, name=f"sH{h}", tag=f"sH{h}", bufs=1)
                nc.tensor.matmul(
                    s,
                    kT[h * DH : (h + 1) * DH, k0 : k0 + 128],
                    qT[h * DH : (h + 1) * DH, q0 : q0 + CHUNK],
                    start=True,
                    stop=True,
                    tile_position=(h * DH, 0),
                )
                ss.append(s)
            return ss

        def attn_exp(ss):
            pa = pPool.tile([128, 2 * CHUNK], BF16, tag="pa")
            for i in range(2):
                sl = slice(i * CHUNK, (i + 1) * CHUNK)
                nc.scalar.activation(
                    pa[:, sl], ss[i], AF.Exp, bias=nshift, scale=1.0
                )
            pb = pPool.tile([128, 2 * CHUNK], I16, tag="pb")
            for i in range(2):
                sl = slice(i * CHUNK, (i + 1) * CHUNK)
                nc.vector.tensor_scalar(
                    pb[:, sl], ss[2 + i], SA, SB, op0=OP.mult, op1=OP.add
                )
            return pa, pb.bitcast(BF16)

        def attn_ctx(kt, p):
            pa, pb = p
            first, last = kt == 0, kt == NKT - 1
            for g, (bank, psrc) in enumerate((("A", pa), ("B", pb))):
                for i in range(2):
                    nc.tensor.matmul(
                        ctx_ps[bank][64 * i : 64 * i + 64, :],
                        v_sb[:, kt, 2 * g + i, :],
                        psrc[:, i * CHUNK : (i + 1) * CHUNK],
                        start=first,
                        stop=last,
                        tile_position=(0, 64 * i),
                        skip_group_check=True,
                    )

        # ---- tails ----
        tail_state = {}

        def tail_copy(qc):
            st = {}
            csA = sbTmp.tile([128, CHUNK], BF16, tag="csA")
            nc.scalar.copy(csA, ctx_ps["A"])
            st["A"] = csA
            csB = sbTmp.tile([128, CHUNK], BF16, tag="csB")
            nc.vector.tensor_copy(csB, ctx_ps["B"])
            st["B"] = csB
            tail_state[qc] = st

        def tail_dps(qc, g):
            # den broadcast via masked bf16 matmul (rows 0/64 -> all rows)
            dps = util_tile(f"dps{g}")
            nc.tensor.matmul(dps, msel, tail_state[qc][g], start=True, stop=True)
            tail_state[qc][g + "d"] = dps

        def tail_recip(qc, g):
            dinv = sbTmp.tile([128, CHUNK], F32, tag=f"di{g}")
            nc.vector.reciprocal_approx_fast(dinv, tail_state[qc][g + "d"])
            tail_state[qc][g + "i"] = dinv

        def tail_mult(qc, g):
            ctxn = sbTmp.tile([128, CHUNK], F32R, tag=f"cn{g}")
            nc.gpsimd.tensor_tensor(
                ctxn, tail_state[qc][g], tail_state[qc][g + "i"], op=OP.mult
            )
            tail_state[qc][g + "n"] = ctxn

        def tail_out(qc):
            q0 = qc * CHUNK
            outp = util_tile("outp")
            for gi, g in enumerate(("A", "B")):
                nc.tensor.matmul(
                    outp,
                    wo_sb[:, gi, :],
                    tail_state[qc][g + "n"],
                    start=(gi == 0),
                    stop=(gi == 1),
                )
            fin = sbTmp.tile([128, CHUNK], F32, tag="fin")
            nc.vector.tensor_add(fin, outp, residT[:, q0 : q0 + CHUNK])
            nc.gpsimd.dma_start(out=outT_d[:, q0 : q0 + CHUNK], in_=fin)

        # ---- schedule ----
        ctx_ps = {
            "A": ps.tile([128, CHUNK], F32, name="ctxA0", tag="ctxA", bufs=1),
            "B": ps.tile([128, CHUNK], F32, name="ctxB0", tag="ctxB", bufs=1),
        }
        drip0 = {
            1: lambda: kproj(1),
            2: lambda: vproj(1),
            4: lambda: kproj(2),
            5: lambda: vproj(2),
            7: lambda: kproj(3),
            8: lambda: vproj(3),
            10: lambda: qproj(1),
            12: lambda: resid_add(0),
            13: lambda: resid_add(1),
        }
        ss = attn_scores(0, 0)
        pending = attn_exp(ss)
        for kt in range(NKT):
            if kt in drip0:
                drip0[kt]()
            if kt + 1 < NKT:
                ss = attn_scores(0, kt + 1)
                nxt = attn_exp(ss)
            else:
                nxt = None
            attn_ctx(kt, pending)
            pending = nxt

        tail_copy(0)

        # chunk 1: chunk-0 tail pieces interleaved
        ctx_ps = {
            "A": ps.tile([128, CHUNK], F32, name="ctxA1", tag="ctxA", bufs=1),
            "B": ps.tile([128, CHUNK], F32, name="ctxB1", tag="ctxB", bufs=1),
        }
        drip1 = {
            1: lambda: tail_dps(0, "A"),
            2: lambda: tail_recip(0, "A"),
            3: lambda: tail_mult(0, "A"),
            5: lambda: tail_dps(0, "B"),
            6: lambda: tail_recip(0, "B"),
            7: lambda: tail_mult(0, "B"),
            9: lambda: tail_out(0),
        }
        ss = attn_scores(1, 0)
        pending = attn_exp(ss)
        for kt in range(NKT):
            if kt in drip1:
                drip1[kt]()
            if kt + 1 < NKT:
                ss = attn_scores(1, kt + 1)
                nxt = attn_exp(ss)
            else:
                nxt = None
            attn_ctx(kt, pending)
            pending = nxt

        # chunk-1 endgame: half-width pipeline so the first out-DMA starts
        # early; output DMAs spread across queues
        q0 = CHUNK
        csA = sbTmp.tile([128, CHUNK], BF16, tag="csA")
        csB = sbTmp.tile([128, CHUNK], BF16, tag="csB")
        diA = sbTmp.tile([128, CHUNK], F32, tag="diA")
        diB = sbTmp.tile([128, CHUNK], F32, tag="diB")
        cnA = sbTmp.tile([128, CHUNK], F32R, tag="cnA")
        cnB = sbTmp.tile([128, CHUNK], F32R, tag="cnB")
        fin = sbTmp.tile([128, CHUNK], F32, tag="fin")
        dps = util_tile("dpsf")
        outp = util_tile("outpf")
        for hf in range(2):
            sl = slice(hf * 256, (hf + 1) * 256)
            nc.scalar.copy(csA[:, sl], ctx_ps["A"][:, sl])
            nc.vector.tensor_copy(csB[:, sl], ctx_ps["B"][:, sl])
            d0 = hf * 256
            nc.tensor.matmul(
                dps[:, d0 : d0 + 256], msel, csA[:, sl], start=True, stop=True
            )
            nc.vector.reciprocal_approx_fast(diA[:, sl], dps[:, d0 : d0 + 256])
            nc.tensor.matmul(
                dps[:, d0 : d0 + 256], msel, csB[:, sl], start=True, stop=True
            )
            nc.vector.reciprocal_approx_fast(diB[:, sl], dps[:, d0 : d0 + 256])
            nc.vector.tensor_mul(cnA[:, sl], csA[:, sl], diA[:, sl])
            nc.vector.tensor_mul(cnB[:, sl], csB[:, sl], diB[:, sl])
            o0 = hf * 256
            nc.tensor.matmul(
                outp[:, o0 : o0 + 256], wo_sb[:, 0, :], cnA[:, sl],
                start=True, stop=False,
            )
            nc.tensor.matmul(
                outp[:, o0 : o0 + 256], wo_sb[:, 1, :], cnB[:, sl],
                start=False, stop=True,
            )
            nc.vector.tensor_add(
                fin[:, sl],
                outp[:, o0 : o0 + 256],
                residT[:, q0 + hf * 256 : q0 + (hf + 1) * 256],
            )
            eng = nc.sync if hf == 0 else nc.gpsimd
            eng.dma_start(
                out=outT_d[:, q0 + hf * 256 : q0 + (hf + 1) * 256], in_=fin[:, sl]
            )

        pPool.release()
        ps.release()
        sbTmp.release()
        sbBig.release()
        sbW.release()
        consts.release()

    nc.compile()
    return nc


def _get_compiled():
    global _compiled
    if _compiled is None:
        _compiled = _build()
    return _compiled


def kernel(x, Wq, bq, Wk, bk, Wv, bv, gamma, beta, Wo, bo):
    bf16 = mybir.dt.np(BF16)
    x = np.asarray(x, dtype=np.float64)
    Wq = np.asarray(Wq, dtype=np.float64)
    Wk = np.asarray(Wk, dtype=np.float64)
    Wv = np.asarray(Wv, dtype=np.float64)
    Wo = np.asarray(Wo, dtype=np.float64)
    gamma = np.asarray(gamma, dtype=np.float64)
    beta = np.asarray(beta, dtype=np.float64)
    bq = np.asarray(bq, dtype=np.float64)
    bk = np.asarray(bk, dtype=np.float64)
    bv = np.asarray(bv, dtype=np.float64)
    bo = np.asarray(bo, dtype=np.float64)

    # fold gamma (and ISQ into q) into the projections; beta into biases
    wq_f = np.ascontiguousarray((Wq * gamma[:, None] * ISQ).astype(bf16))
    wk_f = np.ascontiguousarray((Wk * gamma[:, None]).astype(bf16))
    wv_f = np.ascontiguousarray((Wv * gamma[:, None]).astype(bf16))
    bq_eff = (Wq.T @ beta + bq) * ISQ
    bk_eff = Wk.T @ beta + bk
    bv_eff = Wv.T @ beta + bv
    rbias = Wo.T @ bv_eff + bo

    # Wo rows permuted to the 2-bank ctx layout, shipped pre-transposed
    # as [d, group, e]: group 0 = {h0 at rows 1-33, h1 at 65-97},
    # group 1 = {h2, h3}
    woAB = np.zeros((2, D, D), dtype=np.float64)
    woAB[0, 1:33] = Wo[0 * DH : 1 * DH]
    woAB[0, 65:97] = Wo[1 * DH : 2 * DH]
    woAB[1, 1:33] = Wo[2 * DH : 3 * DH]
    woAB[1, 65:97] = Wo[3 * DH : 4 * DH]
    woAB_t = np.ascontiguousarray(woAB.transpose(1, 0, 2).astype(np.float32))

    vecs_t = np.ascontiguousarray(
        np.stack([bq_eff, bk_eff, rbias], axis=1).astype(np.float32)
    )

    # host LayerNorm in fp64 (gamma/beta folded into weights/biases above)
    mu = x.mean(axis=-1, keepdims=True)
    var = np.square(x - mu).mean(axis=-1, keepdims=True)
    xn = (x - mu) / np.sqrt(var + EPS)  # [B, S, D]

    nc = _get_compiled()

    in_maps = []
    for c in range(N_CORES):
        b, half = c // 2, c % 2
        off = half * QH
        xn_roll = np.roll(xn[b], -off, axis=0)
        x_roll = np.roll(x[b], -off, axis=0)
        in_maps.append(
            {
                "xnT": np.ascontiguousarray(xn_roll.T.astype(bf16)),
                "xt": np.ascontiguousarray(x_roll[0:QH].T.astype(bf16)),
                "wq": wq_f,
                "wk": wk_f,
                "wv": wv_f,
                "woAB": woAB_t,
                "vecs": vecs_t,
            }
        )

    res = run_bass_kernel_spmd(nc, in_maps, core_ids=list(range(N_CORES)), trace=False)

    out = np.empty((B, S, D), dtype=np.float32)
    for c in range(N_CORES):
        b, half = c // 2, c % 2
        off = half * QH
        out[b, off : off + QH, :] = res.results[c]["outT"].T
    return out
